# revision 60
# baseline (speedup 1.0000x reference)
"""GAT + TopKPooling x2 forward on 8 TRN2 NeuronCores.

Data-parallel over the 32-graph batch (4 graphs/core). One SPMD Bass launch
per GAT layer; BatchNorm stats, top-k selection, readouts, and the output
linear run on host between the two launches.

Per layer, nodes are degree-sorted per graph so each 128-node block has a
tight per-dst slot count kb. Slots are node-major: slot j = dst_local*kb + k
lives at (partition j%128, column j//128). The per-slot source features are
pre-gathered ON HOST (gather commutes with the linear projection), uploaded
as per-column lhsT tiles, and the PE computes h|asn per slot directly:
G[:, c, :] = xg_c^T.T @ [W | W@a_s] (head-interleaved columns). Per-slot adn
(dst-side) is also host-computed (adn = x @ (W@a_d)) and uploaded; padding
slots get adn = -200 so their exp() vanishes. Softmax numerators
p = exp(lrelu(asn+adn)) skip the max-subtraction (logits are O(10), softmax is
shift-invariant); the weighted sum + denominator run on the PE as banded 0/1
matmuls accumulating [128, 256+4] per block. DVE does only tensor_tensor ops;
ACT does psum->sbuf copies, exp, and one gelu pass (2 act-table loads total).

No GPSIMD / custom-DMA instructions at all (InstDMAGatherAnt is broken under
this runtime) and no DRAM intermediates inside a launch.
"""

import sys

import numpy as np
import ml_dtypes

import concourse.bacc as bacc
import concourse.mybir as mybir
from concourse.tile import TileContext
from concourse.bass_utils import run_bass_kernel_spmd

B = 32; NPG = 1024; N = B * NPG
EPG = 8192; E = B * EPG
IN = 128; HID = 64; HEADS = 4; F = HID * HEADS; OUT = 256
K1 = 512; K2 = 256
EPS = 1e-5; NEG = 0.2
NC = 8; GPC = B // NC
P = 128
DUM_ADN = -200.0

BF = mybir.dt.bfloat16
FP = mybir.dt.float32
AF = mybir.ActivationFunctionType
BF_NP = ml_dtypes.bfloat16

# interleaved feature order: device col j*4+h = original col h*64+j
ILV = np.array([(j % 4) * 64 + j // 4 for j in range(F)], np.int64)

EXEC_NS = []   # per-launch HW exec time (ns) when NTFF profiling is available
SIM_NS = []    # per-launch TimelineSim estimate (ns)
TRACES = []
BUILT = []     # (name, nc) for offline inspection


# ---------------------------------------------------------------- E tiles ----

def _etile_info(k_list):
    """Banded 0/1 matrices for the per-column slot->dst aggregation matmuls.

    Column c of a kb-block covers slots j = c*128+p; slot j belongs to
    dst-local i = j//kb. E (lhsT of the aggregation) is [128 slots, w dsts],
    zero-padded on the left; full 128-wide for c==0 so start=True clears the
    whole psum block. Tiles are shared across blocks with equal kb."""
    einfo = {}
    eparts = []
    eoff = 0
    p = np.arange(P)
    for kb in sorted(set(k_list)):
        for c in range(kb):
            j0 = c * P
            dl1 = (j0 + P - 1) // kb + 1
            w = P if c == 0 else dl1
            Em = np.zeros((P, w), np.float32)
            Em[p, (j0 + p) // kb] = 1.0
            einfo[(kb, c)] = (eoff, w)
            eparts.append(Em)
            eoff += w
    Ecat = np.concatenate(eparts, 1).astype(BF_NP)
    return einfo, Ecat


# ------------------------------------------------------------- bass module ---

def _build_layer(n, dinb, k_list, einfo, ew, grpb):
    nb = n // P
    sumk = sum(k_list)
    kbmax = max(k_list)
    # max columns per block-group (xgT staging granularity)
    gcmax = max(sum(k_list[g : g + grpb]) for g in range(0, nb, grpb))
    nc = bacc.Bacc("TRN2", target_bir_lowering=False, debug=True)
    xgT = nc.dram_tensor("xgT", [dinb, P, sumk * P], BF, kind="ExternalInput")
    Wp = nc.dram_tensor("Wp", [dinb, P, 260], BF, kind="ExternalInput")
    Ein = nc.dram_tensor("Ein", [P, ew], BF, kind="ExternalInput")
    ads = nc.dram_tensor("ads", [P, sumk * 4], BF, kind="ExternalInput")
    bia = nc.dram_tensor("bia", [P, F + 4], BF, kind="ExternalInput")
    y = nc.dram_tensor("y", [n, F], FP, kind="ExternalOutput")

    with TileContext(nc) as tc:
        with (
            tc.tile_pool(name="cst", bufs=1) as cst,
            tc.tile_pool(name="xg", bufs=2) as xgp,
            tc.tile_pool(name="g", bufs=1) as gp,
            tc.tile_pool(name="sm", bufs=3) as sm,
            tc.tile_pool(name="pg", bufs=2, space="PSUM") as pg,
            tc.tile_pool(name="ps", bufs=2, space="PSUM") as ps,
        ):
            Ws = cst.tile([P, dinb, 260], BF)
            for kc in range(dinb):
                nc.sync.dma_start(Ws[:, kc, :], Wp[kc])
            bias_s = cst.tile([P, F + 4], BF)  # cols 256:260 hold NEG=0.2
            nc.sync.dma_start(bias_s[:], bia[:])
            adn_s = cst.tile([P, sumk, 4], BF)
            Es = cst.tile([P, ew], BF)
            ybuf = cst.tile([P, nb, F], FP)
            for g0 in range(0, nb, grpb):
                gblocks = list(range(g0, min(g0 + grpb, nb)))
                goff = sum(k_list[:g0])
                xgt = xgp.tile([P, dinb, gcmax * P], BF, tag="xg")
                # per-block slices so the PE can start on block g0 before the
                # whole group's xg data has landed
                boff = 0
                for b in gblocks:
                    kb = k_list[b]
                    src0 = (goff + boff) * P
                    for kc in range(dinb):
                        nc.sync.dma_start(
                            xgt[:, kc, boff * P : (boff + kb) * P],
                            xgT[kc, :, src0 : src0 + kb * P])
                    boff += kb
                if g0 == 0:
                    # issued behind the first group's xg slices: needed only
                    # once pass-1b / pass-2 of block 0 start
                    nc.sync.dma_start(adn_s.rearrange("p k h -> p (k h)"),
                                      ads[:])
                    nc.sync.dma_start(Es[:], Ein[:])
                # pass 1 -- projections + softmax numerators for ALL blocks of
                # the group, so pass-2 PE aggregation never stalls on the
                # ACT/DVE chain of the block right before it
                Gs = {}
                off = goff
                for b in gblocks:
                    kb = k_list[b]
                    q0 = off - goff   # column offset within the group tile
                    G = gp.tile([P, kbmax, 260], BF, tag=f"g{b - g0}")
                    Gs[b] = G
                    for c0 in range(0, kb, 3):
                        # per-slot projection: G[:,c,:] = xg_c @ [W|Wa];
                        # 3 columns per 3-bank psum tile -> 1 ACT copy per 3
                        cw = min(3, kb - c0)
                        gps = pg.tile([P, 1536], FP, tag="gp")
                        for ci in range(cw):
                            q = q0 + c0 + ci
                            for kc in range(dinb):
                                nc.tensor.matmul(
                                    gps[:, ci * 512 : ci * 512 + 260],
                                    xgt[:, kc, q * P : (q + 1) * P],
                                    Ws[:, kc, :],
                                    start=(kc == 0), stop=(kc == dinb - 1),
                                )
                        nc.scalar.activation(
                            G[:, c0 : c0 + cw, :],
                            gps.rearrange("p (i x) -> p i x", x=512)[:, 0:cw, 0:260],
                            AF.Copy)
                    lt = sm.tile([P, kbmax, 4], FP, tag="lt")
                    lt2 = sm.tile([P, kbmax, 4], FP, tag="lt2")
                    nc.vector.tensor_add(lt[:, 0:kb, :], G[:, 0:kb, 256:260],
                                         adn_s[:, off : off + kb, :])
                    nc.vector.tensor_mul(
                        lt2[:, 0:kb, :], lt[:, 0:kb, :],
                        bias_s[:, 256:260].unsqueeze(1).to_broadcast([P, kb, 4]))
                    nc.vector.tensor_tensor(
                        out=lt[:, 0:kb, :], in0=lt[:, 0:kb, :],
                        in1=lt2[:, 0:kb, :], op=mybir.AluOpType.max)
                    nc.scalar.activation(G[:, 0:kb, 256:260], lt[:, 0:kb, :],
                                         AF.Exp)
                    h4 = G[:, 0:kb, 0:256].rearrange("p k (j h) -> p k j h", h=4)
                    px = G[:, 0:kb, 256:260].unsqueeze(2).to_broadcast(
                        [P, kb, HID, 4])
                    nc.vector.tensor_mul(h4, h4, px)
                    off += kb
                # pass 2 -- aggregation + normalize
                off = goff
                for b in gblocks:
                    kb = k_list[b]
                    G = Gs[b]
                    yps = ps.tile([P, 512], FP, tag="y")  # one 2KB bank
                    for c in range(kb):
                        eoff, w = einfo[(kb, c)]
                        nc.tensor.matmul(
                            yps[0:w, 0:260], Es[:, eoff : eoff + w],
                            G[:, c, 0:260],
                            start=(c == 0), stop=(c == kb - 1),
                        )
                    rd = sm.tile([P, 4], FP, tag="rd")
                    nc.vector.reciprocal(rd[:], yps[:, 256:260])
                    yv = ybuf[:, b, :].rearrange("p (j h) -> p j h", h=4)
                    nc.vector.tensor_mul(
                        yv, yps[:, 0:256].rearrange("p (j h) -> p j h", h=4),
                        rd.unsqueeze(1).to_broadcast([P, HID, 4]),
                    )
                    off += kb
            # bias+gelu+store in halves, all after the block loop (keeps the
            # Exp-then-Gelu act-table order -> 2 loads) but lets the first
            # half's y DMA overlap the second half's bias/gelu
            ybuf2 = cst.tile([P, nb, F], FP)
            yv = y.rearrange("(b p) f -> p b f", p=P)
            hb = max(nb // 4, 8)
            for h0 in range(0, nb, hb):
                sl = slice(h0, h0 + hb)
                nc.vector.tensor_add(
                    ybuf2[:, sl, :], ybuf[:, sl, :],
                    bias_s[:, 0:F].unsqueeze(1).to_broadcast([P, hb, F]))
                nc.scalar.activation(
                    ybuf[:, sl, :].rearrange("p b f -> p (b f)"),
                    ybuf2[:, sl, :].rearrange("p b f -> p (b f)"), AF.Gelu)
                nc.sync.dma_start(yv[:, sl, :], ybuf[:, sl, :])
    nc.compile()
    return nc


# ------------------------------------------------------------- host: edges ---

def _prep_edges(src, dst, n, npg):
    """src/dst core-local WITH self-loops. Degree-sort nodes per graph.
    Returns perm (new->old), ndeg, ssrc (sources by new dst), starts."""
    deg = np.bincount(dst, minlength=n)
    ngr = n // npg
    perm = np.concatenate([
        g * npg + np.argsort(-deg[g * npg : (g + 1) * npg], kind="stable")
        for g in range(ngr)
    ])
    inv = np.empty(n, np.int64)
    inv[perm] = np.arange(n)
    nsrc = inv[src]
    ndst = inv[dst]
    ndeg = deg[perm]
    order = np.argsort(ndst, kind="stable")
    ssrc = nsrc[order]
    starts = np.zeros(n + 1, np.int64)
    np.cumsum(np.bincount(ndst, minlength=n), out=starts[1:])
    return perm, ndeg, ssrc, starts


def _slot_tables(ndeg, ssrc, starts, n, k_list):
    """Per-slot (source id | dst id | valid) in the node-major slot layout.
    Slot j of block b -> (dst b*128 + j//kb, k = j%kb); slot j = c*128+p
    lands at [p, off+c]."""
    srcs, dsts, vals = [], [], []
    for b, kb in enumerate(k_list):
        j = np.arange(P * kb)
        d = b * P + j // kb
        k = j % kb
        deg = ndeg[d]
        pos = starts[d] + np.minimum(k, np.maximum(deg - 1, 0))
        v = ssrc[pos]
        valid = k < deg
        srcs.append(np.where(valid, v, 0).reshape(kb, P).T)
        dsts.append(d.reshape(kb, P).T)
        vals.append(valid.reshape(kb, P).T)
    cat = lambda xs: np.ascontiguousarray(np.concatenate(xs, 1))
    return cat(srcs), cat(dsts), cat(vals)   # each [P, sumk]


# ----------------------------------------------------------- layer driver ----

def _run_layer(n, dinb, k_list, einfo, ew, grpb, in_maps):
    nc = _build_layer(n, dinb, k_list, einfo, ew, grpb)
    BUILT.append((f"layer_n{n}", nc))
    try:
        from concourse.timeline_sim import TimelineSim
        SIM_NS.append(int(TimelineSim(nc, trace=False).simulate()))
    except Exception:
        pass
    res = run_bass_kernel_spmd(nc, in_maps, core_ids=list(range(NC)))
    if res.exec_time_ns is not None:
        EXEC_NS.append(res.exec_time_ns)
    if res.instructions_and_trace is not None:
        TRACES.append(res.instructions_and_trace[1])
    return [res.results[c]["y"] for c in range(NC)]


def _gat_layer(x_all, src_all, dst_all, n, npg, din, Wm, a_s, a_d, bias):
    """x_all [NC*n, din] fp32 (original feature order), per-core local edges
    WITH self-loops. Returns y_all [NC*n, F] fp32 (original order) after
    bias+gelu."""
    dinb = din // P
    grpb = min(npg // P, 4)   # blocks per xgT staging / 2-pass group
    # fused weight: interleaved W columns + asn columns
    Wi = Wm[:, ILV]
    Wa = np.stack([Wm[:, h * HID : (h + 1) * HID] @ a_s[h] for h in range(HEADS)], 1)
    Wd = np.stack([Wm[:, h * HID : (h + 1) * HID] @ a_d[h] for h in range(HEADS)], 1)
    Wfull = np.concatenate([Wi, Wa], 1)               # [din, 260]
    Wp = np.ascontiguousarray(Wfull.reshape(dinb, P, 260)).astype(BF_NP)
    bia = np.concatenate([
        np.broadcast_to(bias[ILV], (P, F)),
        np.full((P, 4), NEG, np.float32),
    ], 1).astype(BF_NP)

    perms, preps = [], []
    kmax = np.zeros(n // P, np.int64)
    for c in range(NC):
        perm, ndeg, ssrc, starts = _prep_edges(src_all[c], dst_all[c], n, npg)
        perms.append(perm)
        preps.append((ndeg, ssrc, starts))
        kmax = np.maximum(kmax, ndeg.reshape(n // P, P).max(1))
    k_list = [int(max(k, 1)) for k in kmax]
    einfo, Ecat = _etile_info(k_list)
    ew = Ecat.shape[1]
    sumk = sum(k_list)

    in_maps = []
    for c in range(NC):
        xp = x_all[c * n : (c + 1) * n][perms[c]]          # [n, din] fp32
        adn = (xp @ Wd.reshape(din, 4)).astype(np.float32) # [n, 4]
        srcs, dsts, vals = _slot_tables(*preps[c], n, k_list)
        # pre-gathered per-slot source features, transposed per column:
        # xgT[kc, :, q*128+p] = xp[src(q,p), kc*128:(kc+1)*128]
        xg = xp[srcs.T.reshape(-1)] * vals.T.reshape(-1, 1)   # [sumk*128, din]
        xgT = np.ascontiguousarray(
            xg.T.reshape(dinb, P, sumk * P)).astype(BF_NP)
        adn_slot = adn[dsts.T.reshape(-1)]                    # [sumk*128, 4]
        adn_slot[~vals.T.reshape(-1)] = DUM_ADN
        ads = np.ascontiguousarray(
            adn_slot.reshape(sumk, P, 4).transpose(1, 0, 2).reshape(P, sumk * 4)
        ).astype(BF_NP)
        in_maps.append({
            "xgT": xgT, "Wp": Wp, "Ein": Ecat, "ads": ads, "bia": bia,
        })
    try:
        y_cores = _run_layer(n, dinb, k_list, einfo, ew, grpb, in_maps)
        y_all = np.empty((NC * n, F), np.float32)
        for c in range(NC):
            yc = np.asarray(y_cores[c]).astype(np.float32)
            out = np.empty((n, F), np.float32)
            out[perms[c]] = yc
            y_all[c * n : (c + 1) * n][:, ILV] = out   # de-interleave
        return y_all
    except Exception as e:
        print(f"[kernel.py] HW path failed ({type(e).__name__}: {e}); "
              "falling back to numpy", file=sys.stderr)
        return np.concatenate([
            _np_gat(x_all[c * n : (c + 1) * n], src_all[c], dst_all[c],
                    Wm, a_s, a_d, bias)
            for c in range(NC)
        ])


# ------------------------------------------------------- host: numpy parts ---

def _np_gat(xp, src_n, dst_n, W, a_s, a_d, bias):
    from scipy.special import erf
    h = xp @ W
    hh = h.reshape(-1, HEADS, HID)
    asn = np.einsum("nhc,hc->nh", hh, a_s)
    adn = np.einsum("nhc,hc->nh", hh, a_d)
    lg = asn[src_n] + adn[dst_n]
    lg = np.where(lg > 0, lg, NEG * lg)
    p = np.exp(lg)
    den = np.zeros((xp.shape[0], HEADS))
    np.add.at(den, dst_n, p)
    alpha = p / den[dst_n]
    out = np.zeros((xp.shape[0], HEADS, HID))
    np.add.at(out, dst_n, alpha[:, :, None] * hh[src_n])
    out = (out.reshape(-1, F) + bias).astype(np.float32)
    return (out * 0.5 * (1 + erf(out / np.sqrt(2)))).astype(np.float32)


def _bn(x, g, b):
    mu = x.mean(0, dtype=np.float64)
    var = ((x.astype(np.float64) - mu) ** 2).mean(0)
    return ((x - mu) / np.sqrt(var + EPS) * g + b).astype(np.float32)


def _pool_host(x, src, dst, w, n, npg, k):
    score = (x @ w) / np.linalg.norm(w)
    ngr = n // npg
    sc = score.reshape(ngr, npg)
    idx = np.argsort(-sc, axis=1, kind="stable")[:, :k]
    vals = np.take_along_axis(sc, idx, 1)
    gidx = (idx + (np.arange(ngr) * npg)[:, None]).reshape(-1)
    xn = x[gidx] * np.tanh(vals.reshape(-1))[:, None]
    inv = np.full(n, -1, np.int64)
    inv[gidx] = np.arange(ngr * k)
    sn, dn = inv[src], inv[dst]
    valid = (sn >= 0) & (dn >= 0)
    return xn, sn[valid], dn[valid]


def _readout(x, nb, k):
    xr = x.reshape(nb, k, -1)
    return np.concatenate([xr.max(1), xr.mean(1)], axis=1)


# ------------------------------------------------------------------ kernel ---

def kernel(x, edge_index, batch, W1, as1, ad1, b1, g1, be1, pw1,
           W2, as2, ad2, b2, g2, be2, pw2, Wl, bl):
    x = np.asarray(x, np.float32)
    src = np.asarray(edge_index[0], np.int64)
    dst = np.asarray(edge_index[1], np.int64)
    n1 = GPC * NPG
    epc = GPC * EPG

    # ---- layer 1 ----
    loops = np.arange(n1)
    src_c, dst_c = [], []
    for c in range(NC):
        s = src[c * epc : (c + 1) * epc] - c * n1
        d = dst[c * epc : (c + 1) * epc] - c * n1
        src_c.append(np.concatenate([s, loops]))
        dst_c.append(np.concatenate([d, loops]))
    y1 = _gat_layer(x, src_c, dst_c, n1, NPG, IN, np.asarray(W1, np.float32),
                    np.asarray(as1, np.float32), np.asarray(ad1, np.float32),
                    np.asarray(b1, np.float32))
    xbn = _bn(y1, np.asarray(g1, np.float32), np.asarray(be1, np.float32))
    xp, sn, dn = _pool_host(xbn, src, dst, np.asarray(pw1, np.float32), N, NPG, K1)
    x1 = _readout(xp, B, K1)

    # ---- layer 2 ----
    n2 = GPC * K1
    loops2 = np.arange(n2)
    src2_c, dst2_c = [], []
    for c in range(NC):
        m = (sn >= c * n2) & (sn < (c + 1) * n2)
        src2_c.append(np.concatenate([sn[m] - c * n2, loops2]))
        dst2_c.append(np.concatenate([dn[m] - c * n2, loops2]))
    y2 = _gat_layer(xp, src2_c, dst2_c, n2, K1, F, np.asarray(W2, np.float32),
                    np.asarray(as2, np.float32), np.asarray(ad2, np.float32),
                    np.asarray(b2, np.float32))
    xbn2 = _bn(y2, np.asarray(g2, np.float32), np.asarray(be2, np.float32))
    xp2, _, _ = _pool_host(xbn2, sn, dn, np.asarray(pw2, np.float32),
                           B * K1, K1, K2)
    x2 = _readout(xp2, B, K2)

    out = (x1 + x2) @ np.asarray(Wl, np.float32).T + np.asarray(bl, np.float32)
    return out.astype(np.float32)
